# revision 13
# baseline (speedup 1.0000x reference)
"""Trainium2 Bass kernel for nn_CrossAttentionLayer (single-query cross-attention).

Math reformulation (single query token per batch):
    Q[b]      = query[b] @ W_q.T / (sqrt(dk)*TEMP)          # scale folded into W_qT host-side
    qk[b,h,e] = sum_d Q[b,h*64+d] * W_k[h*64+d, e]          # per-head fold of W_k into Q
    scores    = qk[b] @ order[b].T   (contract over e)      # [H, S]
    attn      = softmax(scores)                             # fp32
    ao[b,h,e] = sum_s attn[h,s] * order[b,s,e]              # contract over s
    ctx[b,h*64+d] = sum_e ao[b,h,e] * W_v[h*64+d, e]        # block-diag extract
    y         = LN(ctx @ W_o.T + b_o + query) * gamma + beta
    attn_mean = mean_h attn

This avoids materializing K/V projections (~550 GFLOP -> ~17 GFLOP).
Sharding: data-parallel over batch, 4 batches per core x 8 cores.
Device needs order in both layouts (PE contracts over the partition dim only):
host supplies natural [S,E] and transposed [E,S] copies in bf16.
"""

import sys
import types

for _p in ("/opt/trn_rl_repo",):
    if _p not in sys.path:
        sys.path.insert(0, _p)

# Fallback shim: bass_utils unconditionally imports antenv.axon_hooks when
# trace=True under axon; make sure that import can never crash.
try:
    import antenv.axon_hooks  # noqa: F401
except Exception:
    try:
        import antenv

        _m = types.ModuleType("antenv.axon_hooks")
        _m._hook = None

        def _set(h, _m=_m):
            _m._hook = h

        def _get(_m=_m):
            return _m._hook

        _m.set_axon_ntff_profile_hook = _set
        _m.get_axon_ntff_profile_hook = _get
        sys.modules["antenv.axon_hooks"] = _m
        antenv.axon_hooks = _m
    except Exception:
        pass

import os

import ml_dtypes
import numpy as np

import concourse.bacc as bacc
import concourse.bass as bass
import concourse.tile as tile
from concourse import mybir
from concourse.bass_utils import run_bass_kernel_spmd
from concourse.masks import make_identity

BF = mybir.dt.bfloat16
F32 = mybir.dt.float32
NPBF = ml_dtypes.bfloat16

B, S, E, DM, H, DK = 32, 2048, 2048, 1024, 16, 64
NCORES = 8
BB = B // NCORES  # 4 batches per core
TEMP = 1.5
SCALE = 1.0 / (np.sqrt(DK) * TEMP)
LN_EPS = 1e-5

LAST = {}
_cache = {}


def _build_program():
    STAGE = int(os.environ.get("BASS_KERNEL_STAGE", "6"))
    nc = bacc.Bacc("TRN2", target_bir_lowering=False, debug=False, num_devices=NCORES)
    _emit(nc, STAGE)
    nc.compile()
    return nc


def _emit(nc, STAGE):

    d_qT = nc.dram_tensor("queryT", [E, BB], BF, kind="ExternalInput")
    d_qf = nc.dram_tensor("query_f", [BB, E], F32, kind="ExternalInput")
    d_wqT = nc.dram_tensor("w_qT", [E, DM], BF, kind="ExternalInput")
    d_wk = nc.dram_tensor("w_k", [DM, E], BF, kind="ExternalInput")
    d_wvT = nc.dram_tensor("w_vT", [E, DM], BF, kind="ExternalInput")
    d_woT = nc.dram_tensor("w_oT", [DM, E], BF, kind="ExternalInput")
    d_ga = nc.dram_tensor("gamma4", [BB, E], F32, kind="ExternalInput")
    d_be = nc.dram_tensor("beta4", [BB, E], F32, kind="ExternalInput")
    d_on = nc.dram_tensor("order_n", [BB, S, E], BF, kind="ExternalInput")
    d_ot = nc.dram_tensor("order_t", [BB, E, S], BF, kind="ExternalInput")
    d_y = nc.dram_tensor("out_y", [BB, E], F32, kind="ExternalOutput")
    d_am = nc.dram_tensor("out_am", [BB, S], F32, kind="ExternalOutput")

    with tile.TileContext(nc) as tc:
        with (
            tc.tile_pool(name="consts", bufs=1) as consts,
            tc.tile_pool(name="wts", bufs=1) as wts,
            tc.tile_pool(name="otp", bufs=4) as otp,
            tc.tile_pool(name="onp", bufs=4) as onp,
            tc.tile_pool(name="ab", bufs=2) as ab,
            tc.tile_pool(name="amp", bufs=1) as amp,
            tc.tile_pool(name="ps_big", bufs=1, space="PSUM") as ps_big,
            tc.tile_pool(name="ps_sm", bufs=4, space="PSUM") as ps_sm,
        ):
            ident = consts.tile([128, 128], BF)
            make_identity(nc, ident)

            # ---- weights / small inputs -------------------------------
            wqT_r = d_wqT[:].rearrange("(t p) n -> p t n", p=128)
            wqT = wts.tile([128, 16, DM], BF, tag="wa")
            nc.sync.dma_start(out=wqT[:, 0:4], in_=wqT_r[:, 0:4])
            nc.scalar.dma_start(out=wqT[:, 4:8], in_=wqT_r[:, 4:8])
            nc.sync.dma_start(out=wqT[:, 8:12], in_=wqT_r[:, 8:12])
            nc.scalar.dma_start(out=wqT[:, 12:16], in_=wqT_r[:, 12:16])
            wk_r = d_wk[:].rearrange("(t p) n -> p t n", p=128)
            wk = wts.tile([128, 8, E], BF, tag="wb")
            nc.sync.dma_start(out=wk[:, 0:2], in_=wk_r[:, 0:2])
            nc.scalar.dma_start(out=wk[:, 2:4], in_=wk_r[:, 2:4])
            nc.sync.dma_start(out=wk[:, 4:6], in_=wk_r[:, 4:6])
            nc.scalar.dma_start(out=wk[:, 6:8], in_=wk_r[:, 6:8])
            qT_t = consts.tile([128, 16, BB], BF)
            nc.sync.dma_start(out=qT_t, in_=d_qT[:].rearrange("(t p) n -> p t n", p=128))

            qf = consts.tile([BB, E], F32)
            nc.gpsimd.dma_start(out=qf, in_=d_qf[:])

            def bcast4(dram):
                t = consts.tile([BB, E], F32)
                nc.gpsimd.dma_start(out=t, in_=dram[:])
                return t

            ga4 = bcast4(d_ga)
            be4 = bcast4(d_be)

            eps_t = consts.tile([BB, 1], F32)
            nc.vector.memset(eps_t, LN_EPS)
            ones16 = consts.tile([H, 1], BF)
            nc.vector.memset(ones16, 1.0 / H)

            def probe(ap):
                y_p = consts.tile([BB, E], F32)
                nc.vector.memset(y_p, 0.0)
                if ap is not None:
                    fs = 1
                    for d in ap.shape[1:]:
                        fs *= d
                    nc.vector.tensor_copy(
                        y_p[:, 0:fs], ap.rearrange("p ... -> p (...)") if len(ap.shape) > 2 else ap
                    )
                nc.sync.dma_start(out=d_y[:], in_=y_p[:])

            if STAGE < 1:
                probe(qf)
                return
            # ---- preamble: Q projection -------------------------------
            # Q[b, dm] = sum_e queryT[e, b] * W_qT[e, dm]   (scale pre-folded)
            q_ps = ps_big.tile([BB, 2, 512], F32, tag="big")
            for k in range(16):
                for nb in range(2):
                    nc.tensor.matmul(
                        q_ps[:, nb, :],
                        qT_t[:, k, :],
                        wqT[:, k, nb * 512 : (nb + 1) * 512],
                        start=(k == 0),
                        stop=(k == 15),
                    )
            q_sb = consts.tile([BB, DM], BF)
            nc.vector.tensor_copy(q_sb, q_ps.rearrange("p a b -> p (a b)"))

            # Qblk [dm, (h,b)] block-diagonal: col j=h*4+b holds Q[b, 64h+d]
            # at rows dm = 64h+d, zero elsewhere.
            qblk = consts.tile([128, 8, H * BB], BF)
            nc.vector.memset(qblk, 0.0)
            for t in range(8):
                tp = ps_sm.tile([128, BB], BF, tag="sm")
                nc.tensor.transpose(tp, q_sb[:, t * 128 : (t + 1) * 128], ident[:BB, :BB])
                nc.vector.tensor_copy(
                    qblk[0:64, t, (2 * t) * 4 : (2 * t) * 4 + 4], tp[0:64, :]
                )
                nc.vector.tensor_copy(
                    qblk[64:128, t, (2 * t + 1) * 4 : (2 * t + 1) * 4 + 4], tp[64:128, :]
                )

            if STAGE < 2:
                probe(q_sb)
                return
            # qkT[e, et, h, b] = sum_dm W_k[dm, e] * Qblk[dm, (h,b)]
            qkT = consts.tile([128, 16, H, BB], BF)
            for et in range(16):
                qk_ps = ps_sm.tile([128, H * BB], F32, tag="sm")
                for dmc in range(8):
                    nc.tensor.matmul(
                        qk_ps,
                        wk[:, dmc, et * 128 : (et + 1) * 128],
                        qblk[:, dmc, :],
                        start=(dmc == 0),
                        stop=(dmc == 7),
                    )
                nc.vector.tensor_copy(
                    qkT[:, et].rearrange("p h b -> p (h b)"), qk_ps
                )

            if STAGE < 3:
                probe(qkT[0:BB])
                return

            # tail weights reuse the preamble weight slots (same tags)
            wvT = wts.tile([128, 16, DM], BF, tag="wa")
            nc.gpsimd.dma_start(out=wvT, in_=d_wvT[:].rearrange("(t p) n -> p t n", p=128))
            woT = wts.tile([128, 8, E], BF, tag="wb")
            nc.gpsimd.dma_start(out=woT, in_=d_woT[:].rearrange("(t p) n -> p t n", p=128))

            aoT = consts.tile([128, 16, H, BB], BF)  # [p, et, h, b]

            on_r = d_on[:].rearrange("b (t p) n -> b p t n", p=128)
            ot_r = d_ot[:].rearrange("b (t p) n -> b p t n", p=128)

            # ---- main loop over the core's 4 batches ------------------
            for b in range(BB):
                # pass 1: scores [H, S] = qkT_b.T @ orderT[b]
                sc_ps = ps_big.tile([H, 4, 512], F32, tag="big")
                for kk in range(8):
                    chunk = otp.tile([128, 2, S], BF, tag="ot")
                    (nc.sync, nc.scalar, nc.gpsimd)[kk % 3].dma_start(
                        out=chunk, in_=ot_r[b, :, kk * 2 : kk * 2 + 2, :]
                    )
                    for tt in range(2):
                        k = kk * 2 + tt
                        for nb in range(4):
                            nc.tensor.matmul(
                                sc_ps[:, nb, :],
                                qkT[:, k, :, b],
                                chunk[:, tt, nb * 512 : (nb + 1) * 512],
                                start=(k == 0),
                                stop=(k == 15),
                            )

                # softmax over free dim (s)
                # scores are bounded (|s| <~ 6): exp needs no max-subtraction
                sc_flat = sc_ps.rearrange("p a b -> p (a b)")
                attn_e = ab.tile([H, S], F32, tag="attn_e")
                z_s = ab.tile([H, 1], F32, tag="z")
                nc.scalar.activation(
                    out=attn_e,
                    in_=sc_flat,
                    func=mybir.ActivationFunctionType.Exp,
                    bias=0.0,
                    scale=1.0,
                    accum_out=z_s,
                )
                r_s = ab.tile([H, 1], F32, tag="r")
                nc.vector.reciprocal(r_s, z_s)
                attn_bf = ab.tile([H, S], BF, tag="attn_bf")
                nc.vector.tensor_scalar_mul(attn_bf, attn_e, r_s)

                if STAGE < 4:
                    continue
                # attn transposed for pass 2: [s, h] tiles (PE critical path)
                attnT = ab.tile([128, 16, H], BF, tag="attnT")
                for t in range(16):
                    tp = ps_sm.tile([128, H], BF, tag="sm")
                    nc.tensor.transpose(
                        tp, attn_bf[:, t * 128 : (t + 1) * 128], ident[:H, :H]
                    )
                    nc.vector.tensor_copy(attnT[:, t, :], tp)

                # attn_mean row: (ones/16).T @ attn_bf on PE (fills DMA waits)
                am_row = amp.tile([1, S], F32, tag="am_row")
                for nb in range(4):
                    am_ps = ps_sm.tile([1, 512], F32, tag="sm")
                    nc.tensor.matmul(
                        am_ps,
                        ones16,
                        attn_bf[:, nb * 512 : (nb + 1) * 512],
                        start=True,
                        stop=True,
                    )
                    nc.scalar.copy(am_row[:, nb * 512 : (nb + 1) * 512], am_ps)
                nc.gpsimd.dma_start(out=d_am[b : b + 1, :], in_=am_row)

                if STAGE < 5:
                    continue
                # pass 2: ao [H, E] = attn @ order[b]
                ao_ps = ps_big.tile([H, 4, 512], F32, tag="big")
                for kk in range(8):
                    chunk = onp.tile([128, 2, E], BF, tag="on")
                    (nc.sync, nc.scalar, nc.gpsimd)[kk % 3].dma_start(
                        out=chunk, in_=on_r[b, :, kk * 2 : kk * 2 + 2, :]
                    )
                    for tt in range(2):
                        k = kk * 2 + tt
                        for nb in range(4):
                            nc.tensor.matmul(
                                ao_ps[:, nb, :],
                                attnT[:, k, :],
                                chunk[:, tt, nb * 512 : (nb + 1) * 512],
                                start=(k == 0),
                                stop=(k == 15),
                            )

                ao_sb = ab.tile([H, E], BF, tag="ao_sb")
                nc.vector.tensor_copy(ao_sb, ao_ps.rearrange("p a b -> p (a b)"))
                for t in range(16):
                    tp = ps_sm.tile([128, H], BF, tag="sm")
                    nc.tensor.transpose(
                        tp, ao_sb[:, t * 128 : (t + 1) * 128], ident[:H, :H]
                    )
                    nc.vector.tensor_copy(aoT[:, t, :, b], tp)

            if STAGE < 4:
                probe(attn_e[0:BB])
                return
            if STAGE < 5:
                probe(attnT[0:BB])
                return
            if STAGE < 6:
                probe(aoT[0:BB])
                return

            # ---- tail: ctx, W_o projection, residual + LN -------------
            # ctx_full[(h,b), dm] = sum_e aoT[e, (h,b)] * W_vT[e, dm]
            ctx_ps = ps_big.tile([H * BB, 2, 512], F32, tag="big")
            for k in range(16):
                for nb in range(2):
                    nc.tensor.matmul(
                        ctx_ps[:, nb, :],
                        aoT[:, k, :, :],
                        wvT[:, k, nb * 512 : (nb + 1) * 512],
                        start=(k == 0),
                        stop=(k == 15),
                    )
            ctx_sb = consts.tile([H * BB, DM], BF)
            nc.vector.tensor_copy(ctx_sb, ctx_ps.rearrange("p a b -> p (a b)"))

            # transpose ctx_full [64, 1024] -> [1024, 64] tiles, then gather the
            # block-diagonal columns (dm row p of tile t belongs to head
            # h = 2t + (p >= 64); its 4 batch columns are h*4..h*4+4).
            ctxT = consts.tile([128, 8, BB], BF)
            for t in range(8):
                tp = ps_sm.tile([128, H * BB], BF, tag="sm")
                nc.tensor.transpose(
                    tp, ctx_sb[:, t * 128 : (t + 1) * 128], ident[: H * BB, : H * BB]
                )
                nc.vector.tensor_copy(ctxT[0:64, t, :], tp[0:64, 8 * t : 8 * t + 4])
                nc.vector.tensor_copy(
                    ctxT[64:128, t, :], tp[64:128, 8 * t + 4 : 8 * t + 8]
                )

            # out[b, e] = sum_dm ctxT[dm, b] * W_oT[dm, e]
            out_ps = ps_big.tile([BB, 4, 512], F32, tag="big")
            for k in range(8):
                for nb in range(4):
                    nc.tensor.matmul(
                        out_ps[:, nb, :],
                        ctxT[:, k, :],
                        woT[:, k, nb * 512 : (nb + 1) * 512],
                        start=(k == 0),
                        stop=(k == 7),
                    )

            x_s = consts.tile([BB, E], F32)
            nc.vector.tensor_add(x_s, out_ps.rearrange("p a b -> p (a b)"), qf)

            # layernorm over free dim
            nsub = E // 512
            stats = consts.tile([BB, nsub, 6], F32)
            for g in range(nsub):
                nc.vector.bn_stats(stats[:, g, :], x_s[:, g * 512 : (g + 1) * 512])
            mv = consts.tile([BB, 2], F32)
            nc.vector.bn_aggr(mv, stats)
            rstd = consts.tile([BB, 1], F32)
            nc.scalar.activation(
                out=rstd,
                in_=mv[:, 1:2],
                func=mybir.ActivationFunctionType.Sqrt,
                bias=eps_t,
                scale=1.0,
            )
            nc.vector.reciprocal(rstd, rstd)
            nc.vector.tensor_scalar(
                out=x_s,
                in0=x_s,
                scalar1=mv[:, 0:1],
                scalar2=rstd,
                op0=mybir.AluOpType.subtract,
                op1=mybir.AluOpType.mult,
            )
            nc.vector.tensor_mul(x_s, x_s, ga4)
            nc.vector.tensor_add(x_s, x_s, be4)

            nc.gpsimd.dma_start(out=d_y[:], in_=x_s[:])


def _get_program():
    if "nc" not in _cache:
        _cache["nc"] = _build_program()
    return _cache["nc"]


def kernel(**inputs):
    q = np.asarray(inputs["query"], np.float32)
    order = np.asarray(inputs["order"], np.float32)
    W_q = np.asarray(inputs["W_q"], np.float32)
    W_k = np.asarray(inputs["W_k"], np.float32)
    W_v = np.asarray(inputs["W_v"], np.float32)
    W_o = np.asarray(inputs["W_o"], np.float32)
    b_o = np.asarray(inputs["b_o"], np.float32)
    gamma = np.asarray(inputs["gamma"], np.float32)
    beta = np.asarray(inputs["beta"], np.float32)

    wqT_np = np.ascontiguousarray(W_q.T * SCALE).astype(NPBF)
    wk_np = np.ascontiguousarray(W_k).astype(NPBF)
    wvT_np = np.ascontiguousarray(W_v.T).astype(NPBF)
    woT_np = np.ascontiguousarray(W_o.T).astype(NPBF)

    nc = _get_program()

    in_maps = []
    for c in range(NCORES):
        sl = slice(c * BB, (c + 1) * BB)
        ob = order[sl].astype(NPBF)
        in_maps.append(
            {
                "queryT": np.ascontiguousarray(q[sl].T).astype(NPBF),
                "query_f": np.ascontiguousarray(q[sl] + b_o[None, :]),
                "w_qT": wqT_np,
                "w_k": wk_np,
                "w_vT": wvT_np,
                "w_oT": woT_np,
                "gamma4": np.ascontiguousarray(np.tile(gamma[None, :], (BB, 1))),
                "beta4": np.ascontiguousarray(np.tile(beta[None, :], (BB, 1))),
                "order_n": ob,
                "order_t": np.ascontiguousarray(ob.transpose(0, 2, 1)),
            }
        )

    trace = os.environ.get("BASS_KERNEL_TRACE", "0") == "1"
    res = run_bass_kernel_spmd(nc, in_maps, list(range(NCORES)), trace=trace)
    LAST["exec_time_ns"] = res.exec_time_ns
    LAST["mean_exec_time_ns"] = res.mean_exec_time_ns
    LAST["results"] = res

    y = np.concatenate([r["out_y"] for r in res.results], axis=0)
    am = np.concatenate([r["out_am"] for r in res.results], axis=0)
    return (y.astype(np.float32), am.astype(np.float32))


# revision 14
# speedup vs baseline: 1.1194x; 1.1194x over previous
"""Trainium2 Bass kernel for nn_CrossAttentionLayer (single-query cross-attention).

Math reformulation (single query token per batch):
    Q[b]      = query[b] @ W_q.T / (sqrt(dk)*TEMP)          # scale folded into W_qT host-side
    qk[b,h,e] = sum_d Q[b,h*64+d] * W_k[h*64+d, e]          # per-head fold of W_k into Q
    scores    = qk[b] @ order[b].T   (contract over e)      # [H, S]
    attn      = softmax(scores)                             # fp32
    ao[b,h,e] = sum_s attn[h,s] * order[b,s,e]              # contract over s
    ctx[b,h*64+d] = sum_e ao[b,h,e] * W_v[h*64+d, e]        # block-diag extract
    y         = LN(ctx @ W_o.T + b_o + query) * gamma + beta
    attn_mean = mean_h attn

This avoids materializing K/V projections (~550 GFLOP -> ~17 GFLOP).
Sharding: data-parallel over batch, 4 batches per core x 8 cores.
Device needs order in both layouts (PE contracts over the partition dim only):
host supplies natural [S,E] and transposed [E,S] copies in bf16.
"""

import sys
import types

for _p in ("/opt/trn_rl_repo",):
    if _p not in sys.path:
        sys.path.insert(0, _p)

# Fallback shim: bass_utils unconditionally imports antenv.axon_hooks when
# trace=True under axon; make sure that import can never crash.
try:
    import antenv.axon_hooks  # noqa: F401
except Exception:
    try:
        import antenv

        _m = types.ModuleType("antenv.axon_hooks")
        _m._hook = None

        def _set(h, _m=_m):
            _m._hook = h

        def _get(_m=_m):
            return _m._hook

        _m.set_axon_ntff_profile_hook = _set
        _m.get_axon_ntff_profile_hook = _get
        sys.modules["antenv.axon_hooks"] = _m
        antenv.axon_hooks = _m
    except Exception:
        pass

import os

import ml_dtypes
import numpy as np

import concourse.bacc as bacc
import concourse.bass as bass
import concourse.tile as tile
from concourse import mybir
from concourse.bass_utils import run_bass_kernel_spmd
from concourse.masks import make_identity

BF = mybir.dt.bfloat16
F32 = mybir.dt.float32
NPBF = ml_dtypes.bfloat16

B, S, E, DM, H, DK = 32, 2048, 2048, 1024, 16, 64
NCORES = 8
BB = B // NCORES  # 4 batches per core
TEMP = 1.5
SCALE = 1.0 / (np.sqrt(DK) * TEMP)
LN_EPS = 1e-5

LAST = {}
_cache = {}


def _build_program():
    STAGE = int(os.environ.get("BASS_KERNEL_STAGE", "6"))
    nc = bacc.Bacc("TRN2", target_bir_lowering=False, debug=False, num_devices=NCORES)
    _emit(nc, STAGE)
    nc.compile()
    return nc


def _emit(nc, STAGE):

    d_qT = nc.dram_tensor("queryT", [E, BB], BF, kind="ExternalInput")
    d_qf = nc.dram_tensor("query_f", [BB, E], F32, kind="ExternalInput")
    d_wqT = nc.dram_tensor("w_qT", [E, DM], BF, kind="ExternalInput")
    d_wk = nc.dram_tensor("w_k", [DM, E], BF, kind="ExternalInput")
    d_wvT = nc.dram_tensor("w_vT", [E, DM], BF, kind="ExternalInput")
    d_woT = nc.dram_tensor("w_oT", [DM, E], BF, kind="ExternalInput")
    d_ga = nc.dram_tensor("gamma4", [BB, E], F32, kind="ExternalInput")
    d_be = nc.dram_tensor("beta4", [BB, E], F32, kind="ExternalInput")
    d_on = nc.dram_tensor("order_n", [BB, S, E], BF, kind="ExternalInput")
    d_ot = nc.dram_tensor("order_t", [BB, E, S], BF, kind="ExternalInput")
    d_y = nc.dram_tensor("out_y", [BB, E], F32, kind="ExternalOutput")
    d_am = nc.dram_tensor("out_am", [BB, S], F32, kind="ExternalOutput")

    with tile.TileContext(nc) as tc:
        with (
            tc.tile_pool(name="consts", bufs=1) as consts,
            tc.tile_pool(name="wts", bufs=1) as wts,
            tc.tile_pool(name="otp", bufs=4) as otp,
            tc.tile_pool(name="onp", bufs=4) as onp,
            tc.tile_pool(name="ab", bufs=2) as ab,
            tc.tile_pool(name="amp", bufs=1) as amp,
            tc.tile_pool(name="ps_big", bufs=1, space="PSUM") as ps_big,
            tc.tile_pool(name="ps_sm", bufs=4, space="PSUM") as ps_sm,
        ):
            ident = consts.tile([128, 128], BF)
            make_identity(nc, ident)

            # ---- weights / small inputs -------------------------------
            wqT_r = d_wqT[:].rearrange("(t p) n -> p t n", p=128)
            wqT = wts.tile([128, 16, DM], BF, tag="wa")
            nc.sync.dma_start(out=wqT[:, 0:4], in_=wqT_r[:, 0:4])
            nc.scalar.dma_start(out=wqT[:, 4:8], in_=wqT_r[:, 4:8])
            nc.sync.dma_start(out=wqT[:, 8:12], in_=wqT_r[:, 8:12])
            nc.scalar.dma_start(out=wqT[:, 12:16], in_=wqT_r[:, 12:16])
            wk_r = d_wk[:].rearrange("(t p) n -> p t n", p=128)
            wk = wts.tile([128, 8, E], BF, tag="wb")
            nc.sync.dma_start(out=wk[:, 0:2], in_=wk_r[:, 0:2])
            nc.scalar.dma_start(out=wk[:, 2:4], in_=wk_r[:, 2:4])
            nc.sync.dma_start(out=wk[:, 4:6], in_=wk_r[:, 4:6])
            nc.scalar.dma_start(out=wk[:, 6:8], in_=wk_r[:, 6:8])
            qT_t = consts.tile([128, 16, BB], BF)
            nc.sync.dma_start(out=qT_t, in_=d_qT[:].rearrange("(t p) n -> p t n", p=128))

            qf = consts.tile([BB, E], F32)
            nc.gpsimd.dma_start(out=qf, in_=d_qf[:])

            def bcast4(dram):
                t = consts.tile([BB, E], F32)
                nc.gpsimd.dma_start(out=t, in_=dram[:])
                return t

            ga4 = bcast4(d_ga)
            be4 = bcast4(d_be)

            eps_t = consts.tile([BB, 1], F32)
            nc.vector.memset(eps_t, LN_EPS)
            ones16 = consts.tile([H, 1], BF)
            nc.vector.memset(ones16, 1.0 / H)

            def probe(ap):
                y_p = consts.tile([BB, E], F32)
                nc.vector.memset(y_p, 0.0)
                if ap is not None:
                    fs = 1
                    for d in ap.shape[1:]:
                        fs *= d
                    nc.vector.tensor_copy(
                        y_p[:, 0:fs], ap.rearrange("p ... -> p (...)") if len(ap.shape) > 2 else ap
                    )
                nc.sync.dma_start(out=d_y[:], in_=y_p[:])

            if STAGE < 1:
                probe(qf)
                return
            # ---- preamble: Q projection -------------------------------
            # Q[b, dm] = sum_e queryT[e, b] * W_qT[e, dm]   (scale pre-folded)
            q_ps = ps_big.tile([BB, 2, 512], F32, tag="big")
            for k in range(16):
                for nb in range(2):
                    nc.tensor.matmul(
                        q_ps[:, nb, :],
                        qT_t[:, k, :],
                        wqT[:, k, nb * 512 : (nb + 1) * 512],
                        start=(k == 0),
                        stop=(k == 15),
                    )
            q_sb = consts.tile([BB, DM], BF)
            nc.vector.tensor_copy(q_sb, q_ps.rearrange("p a b -> p (a b)"))

            # Qblk [dm, (h,b)] block-diagonal: col j=h*4+b holds Q[b, 64h+d]
            # at rows dm = 64h+d, zero elsewhere.
            qblk = consts.tile([128, 8, H * BB], BF)
            nc.vector.memset(qblk, 0.0)
            for t in range(8):
                tp = ps_sm.tile([128, BB], BF, tag="sm")
                nc.tensor.transpose(tp, q_sb[:, t * 128 : (t + 1) * 128], ident[:BB, :BB])
                nc.vector.tensor_copy(
                    qblk[0:64, t, (2 * t) * 4 : (2 * t) * 4 + 4], tp[0:64, :]
                )
                nc.vector.tensor_copy(
                    qblk[64:128, t, (2 * t + 1) * 4 : (2 * t + 1) * 4 + 4], tp[64:128, :]
                )

            if STAGE < 2:
                probe(q_sb)
                return
            # qkT[e, et, h, b] = sum_dm W_k[dm, e] * Qblk[dm, (h,b)]
            qkT = consts.tile([128, 16, H, BB], BF)
            for et in range(16):
                qk_ps = ps_sm.tile([128, H * BB], F32, tag="sm")
                for dmc in range(8):
                    nc.tensor.matmul(
                        qk_ps,
                        wk[:, dmc, et * 128 : (et + 1) * 128],
                        qblk[:, dmc, :],
                        start=(dmc == 0),
                        stop=(dmc == 7),
                    )
                nc.vector.tensor_copy(
                    qkT[:, et].rearrange("p h b -> p (h b)"), qk_ps
                )

            if STAGE < 3:
                probe(qkT[0:BB])
                return

            # tail weights reuse the preamble weight slots (same tags)
            wvT = wts.tile([128, 16, DM], BF, tag="wa")
            nc.gpsimd.dma_start(out=wvT, in_=d_wvT[:].rearrange("(t p) n -> p t n", p=128))
            woT = wts.tile([128, 8, E], BF, tag="wb")
            nc.gpsimd.dma_start(out=woT, in_=d_woT[:].rearrange("(t p) n -> p t n", p=128))

            aoT = consts.tile([128, 16, H, BB], BF)  # [p, et, h, b]

            on_r = d_on[:].rearrange("b (t p) n -> b p t n", p=128)
            ot_r = d_ot[:].rearrange("b (t p) n -> b p t n", p=128)

            # ---- main loop over the core's 4 batches ------------------
            for b in range(BB):
                # pass 1: scores [H, S] = qkT_b.T @ orderT[b]
                sc_ps = ps_big.tile([H, 4, 512], F32, tag="big")
                for kk in range(8):
                    chunk = otp.tile([128, 2, S], BF, tag="ot")
                    (nc.sync if kk % 2 == 0 else nc.scalar).dma_start(
                        out=chunk, in_=ot_r[b, :, kk * 2 : kk * 2 + 2, :]
                    )
                    for tt in range(2):
                        k = kk * 2 + tt
                        for nb in range(4):
                            nc.tensor.matmul(
                                sc_ps[:, nb, :],
                                qkT[:, k, :, b],
                                chunk[:, tt, nb * 512 : (nb + 1) * 512],
                                start=(k == 0),
                                stop=(k == 15),
                            )

                # softmax over free dim (s)
                # scores are bounded (|s| <~ 6): exp needs no max-subtraction
                sc_flat = sc_ps.rearrange("p a b -> p (a b)")
                attn_e = ab.tile([H, S], F32, tag="attn_e")
                z_s = ab.tile([H, 1], F32, tag="z")
                nc.scalar.activation(
                    out=attn_e,
                    in_=sc_flat,
                    func=mybir.ActivationFunctionType.Exp,
                    bias=0.0,
                    scale=1.0,
                    accum_out=z_s,
                )
                r_s = ab.tile([H, 1], F32, tag="r")
                nc.vector.reciprocal(r_s, z_s)
                attn_bf = ab.tile([H, S], BF, tag="attn_bf")
                nc.vector.tensor_scalar_mul(attn_bf, attn_e, r_s)

                if STAGE < 4:
                    continue
                # attn transposed for pass 2: [s, h] tiles (PE critical path)
                attnT = ab.tile([128, 16, H], BF, tag="attnT")
                for t in range(16):
                    tp = ps_sm.tile([128, H], BF, tag="sm")
                    nc.tensor.transpose(
                        tp, attn_bf[:, t * 128 : (t + 1) * 128], ident[:H, :H]
                    )
                    nc.vector.tensor_copy(attnT[:, t, :], tp)

                # attn_mean row: (ones/16).T @ attn_bf on PE (fills DMA waits)
                am_row = amp.tile([1, S], F32, tag="am_row")
                for nb in range(4):
                    am_ps = ps_sm.tile([1, 512], F32, tag="sm")
                    nc.tensor.matmul(
                        am_ps,
                        ones16,
                        attn_bf[:, nb * 512 : (nb + 1) * 512],
                        start=True,
                        stop=True,
                    )
                    nc.scalar.copy(am_row[:, nb * 512 : (nb + 1) * 512], am_ps)
                nc.gpsimd.dma_start(out=d_am[b : b + 1, :], in_=am_row)

                if STAGE < 5:
                    continue
                # pass 2: ao [H, E] = attn @ order[b]
                ao_ps = ps_big.tile([H, 4, 512], F32, tag="big")
                for kk in range(8):
                    chunk = onp.tile([128, 2, E], BF, tag="on")
                    (nc.sync if kk % 2 == 0 else nc.scalar).dma_start(
                        out=chunk, in_=on_r[b, :, kk * 2 : kk * 2 + 2, :]
                    )
                    for tt in range(2):
                        k = kk * 2 + tt
                        for nb in range(4):
                            nc.tensor.matmul(
                                ao_ps[:, nb, :],
                                attnT[:, k, :],
                                chunk[:, tt, nb * 512 : (nb + 1) * 512],
                                start=(k == 0),
                                stop=(k == 15),
                            )

                ao_sb = ab.tile([H, E], BF, tag="ao_sb")
                nc.vector.tensor_copy(ao_sb, ao_ps.rearrange("p a b -> p (a b)"))
                for t in range(16):
                    tp = ps_sm.tile([128, H], BF, tag="sm")
                    nc.tensor.transpose(
                        tp, ao_sb[:, t * 128 : (t + 1) * 128], ident[:H, :H]
                    )
                    nc.vector.tensor_copy(aoT[:, t, :, b], tp)

            if STAGE < 4:
                probe(attn_e[0:BB])
                return
            if STAGE < 5:
                probe(attnT[0:BB])
                return
            if STAGE < 6:
                probe(aoT[0:BB])
                return

            # ---- tail: ctx, W_o projection, residual + LN -------------
            # ctx_full[(h,b), dm] = sum_e aoT[e, (h,b)] * W_vT[e, dm]
            ctx_ps = ps_big.tile([H * BB, 2, 512], F32, tag="big")
            for k in range(16):
                for nb in range(2):
                    nc.tensor.matmul(
                        ctx_ps[:, nb, :],
                        aoT[:, k, :, :],
                        wvT[:, k, nb * 512 : (nb + 1) * 512],
                        start=(k == 0),
                        stop=(k == 15),
                    )
            ctx_sb = consts.tile([H * BB, DM], BF)
            nc.vector.tensor_copy(ctx_sb, ctx_ps.rearrange("p a b -> p (a b)"))

            # transpose ctx_full [64, 1024] -> [1024, 64] tiles, then gather the
            # block-diagonal columns (dm row p of tile t belongs to head
            # h = 2t + (p >= 64); its 4 batch columns are h*4..h*4+4).
            ctxT = consts.tile([128, 8, BB], BF)
            for t in range(8):
                tp = ps_sm.tile([128, H * BB], BF, tag="sm")
                nc.tensor.transpose(
                    tp, ctx_sb[:, t * 128 : (t + 1) * 128], ident[: H * BB, : H * BB]
                )
                nc.vector.tensor_copy(ctxT[0:64, t, :], tp[0:64, 8 * t : 8 * t + 4])
                nc.vector.tensor_copy(
                    ctxT[64:128, t, :], tp[64:128, 8 * t + 4 : 8 * t + 8]
                )

            # out[b, e] = sum_dm ctxT[dm, b] * W_oT[dm, e]
            out_ps = ps_big.tile([BB, 4, 512], F32, tag="big")
            for k in range(8):
                for nb in range(4):
                    nc.tensor.matmul(
                        out_ps[:, nb, :],
                        ctxT[:, k, :],
                        woT[:, k, nb * 512 : (nb + 1) * 512],
                        start=(k == 0),
                        stop=(k == 7),
                    )

            x_s = consts.tile([BB, E], F32)
            nc.vector.tensor_add(x_s, out_ps.rearrange("p a b -> p (a b)"), qf)

            # layernorm over free dim
            nsub = E // 512
            stats = consts.tile([BB, nsub, 6], F32)
            for g in range(nsub):
                nc.vector.bn_stats(stats[:, g, :], x_s[:, g * 512 : (g + 1) * 512])
            mv = consts.tile([BB, 2], F32)
            nc.vector.bn_aggr(mv, stats)
            rstd = consts.tile([BB, 1], F32)
            nc.scalar.activation(
                out=rstd,
                in_=mv[:, 1:2],
                func=mybir.ActivationFunctionType.Sqrt,
                bias=eps_t,
                scale=1.0,
            )
            nc.vector.reciprocal(rstd, rstd)
            nc.vector.tensor_scalar(
                out=x_s,
                in0=x_s,
                scalar1=mv[:, 0:1],
                scalar2=rstd,
                op0=mybir.AluOpType.subtract,
                op1=mybir.AluOpType.mult,
            )
            nc.vector.tensor_mul(x_s, x_s, ga4)
            nc.vector.tensor_add(x_s, x_s, be4)

            nc.gpsimd.dma_start(out=d_y[:], in_=x_s[:])


def _get_program():
    if "nc" not in _cache:
        _cache["nc"] = _build_program()
    return _cache["nc"]


def kernel(**inputs):
    q = np.asarray(inputs["query"], np.float32)
    order = np.asarray(inputs["order"], np.float32)
    W_q = np.asarray(inputs["W_q"], np.float32)
    W_k = np.asarray(inputs["W_k"], np.float32)
    W_v = np.asarray(inputs["W_v"], np.float32)
    W_o = np.asarray(inputs["W_o"], np.float32)
    b_o = np.asarray(inputs["b_o"], np.float32)
    gamma = np.asarray(inputs["gamma"], np.float32)
    beta = np.asarray(inputs["beta"], np.float32)

    wqT_np = np.ascontiguousarray(W_q.T * SCALE).astype(NPBF)
    wk_np = np.ascontiguousarray(W_k).astype(NPBF)
    wvT_np = np.ascontiguousarray(W_v.T).astype(NPBF)
    woT_np = np.ascontiguousarray(W_o.T).astype(NPBF)

    nc = _get_program()

    in_maps = []
    for c in range(NCORES):
        sl = slice(c * BB, (c + 1) * BB)
        ob = order[sl].astype(NPBF)
        in_maps.append(
            {
                "queryT": np.ascontiguousarray(q[sl].T).astype(NPBF),
                "query_f": np.ascontiguousarray(q[sl] + b_o[None, :]),
                "w_qT": wqT_np,
                "w_k": wk_np,
                "w_vT": wvT_np,
                "w_oT": woT_np,
                "gamma4": np.ascontiguousarray(np.tile(gamma[None, :], (BB, 1))),
                "beta4": np.ascontiguousarray(np.tile(beta[None, :], (BB, 1))),
                "order_n": ob,
                "order_t": np.ascontiguousarray(ob.transpose(0, 2, 1)),
            }
        )

    trace = os.environ.get("BASS_KERNEL_TRACE", "0") == "1"
    res = run_bass_kernel_spmd(nc, in_maps, list(range(NCORES)), trace=trace)
    LAST["exec_time_ns"] = res.exec_time_ns
    LAST["mean_exec_time_ns"] = res.mean_exec_time_ns
    LAST["results"] = res

    y = np.concatenate([r["out_y"] for r in res.results], axis=0)
    am = np.concatenate([r["out_am"] for r in res.results], axis=0)
    return (y.astype(np.float32), am.astype(np.float32))
